# revision 8
# baseline (speedup 1.0000x reference)
"""Trainium2 Bass kernel for nn_ConcentrationPredictor.

Reaction-diffusion RK4 integrator: dc/dt = D0 * ret(c) * lap(c), where
ret(c) = sigmoid(MLP(tanh x3)) evaluated pointwise, lap is a 1-D 3-point
stencil with a Dirichlet-flux left BC and Cauchy right BC. 33 time points,
32 RK4 steps, N = 65536 grid points.

Distribution: 1-D domain decomposition over 8 NeuronCores with 128-point
ghost zones on each side. RK4 widens the dependency stencil by 4 points per
step, so 32 steps * 4 = 128 ghost points allow each core to integrate its
slab completely independently -- zero inter-core communication. Each core's
slab is 8192 + 2*128 = 8448 points (edge cores get one-sided ghosts). The
fake boundary condition applied at interior slab edges contaminates at most
4*t points by step t, never reaching the owned region.

Per-core layout: the slab lives "tall" as [128 partitions x 66] f32 (grid
index g = 66*p + j) for the stencil (free-dim shifts) and DVE/ACT work at
full 128-lane occupancy. The MLP (1-15-15-15-1) runs "packed" as a
block-diagonal batch of 8 MLPs across 120 partitions: block r handles
chunks 16r..16r+15 of the grid, so a [120,120] block-diag matmul advances
8192+ points per 15-wide layer. Fold tall->packed is 16 partition-strided
SBUF->SBUF DMAs; unfold packed->tall is fused into the last MLP layer as 16
accumulating matmuls with scatter weight matrices (B_q), so the sigmoid
lands directly in tall layout.
"""

import os
import sys

sys.path.insert(0, "/opt/trn_rl_repo")

import numpy as np

N_FULL = 65536
T_FULL = 33
NCORES = 8
GHOST = 128
L = 8448           # slab length per core
WT = 66            # tall width  (L / 128)
BW = 1056          # packed block width (L / 8)
OWN = N_FULL // NCORES  # 8192

DX = np.float32(0.04)
D0 = np.float32(0.0005 / 0.04 ** 2)   # 0.3125
DDX = np.float32(D0 * DX)             # Cauchy BC factor
BC00 = 1.0

_CACHE = {}


def _build(nsteps, cf2, cf4, gam, use_f32r=True):
    import concourse.bacc as bacc
    import concourse.tile as tile
    import concourse.mybir as mybir

    dt = mybir.dt
    AF = mybir.ActivationFunctionType
    OP = mybir.AluOpType

    nc = bacc.Bacc("TRN2", target_bir_lowering=False, debug=False,
                   num_devices=NCORES)

    c0s = nc.dram_tensor("c0s", [L], dt.float32, kind="ExternalInput").ap()
    w1s = nc.dram_tensor("w1s", [8, 120], dt.float32, kind="ExternalInput").ap()
    w2s = nc.dram_tensor("w2s", [120, 120], dt.float32, kind="ExternalInput").ap()
    w3s = nc.dram_tensor("w3s", [120, 120], dt.float32, kind="ExternalInput").ap()
    bqd = nc.dram_tensor("bq", [120, 2048], dt.float32, kind="ExternalInput").ap()
    b1d = nc.dram_tensor("b1", [120, 1], dt.float32, kind="ExternalInput").ap()
    b2d = nc.dram_tensor("b2", [120, 1], dt.float32, kind="ExternalInput").ap()
    b3d = nc.dram_tensor("b3", [120, 1], dt.float32, kind="ExternalInput").ap()
    b4d = nc.dram_tensor("b4", [128, 1], dt.float32, kind="ExternalInput").ap()
    outs = nc.dram_tensor("outs", [nsteps, L], dt.float32,
                          kind="ExternalOutput").ap()

    def mmdt(ap):
        return ap.bitcast(dt.float32r) if use_f32r else ap

    with tile.TileContext(nc) as tc:
        with tc.tile_pool(name="consts", bufs=1) as cp, \
             tc.tile_pool(name="state", bufs=1) as sp, \
             tc.tile_pool(name="work", bufs=2) as wp, \
             tc.tile_pool(name="psA", bufs=1, space="PSUM") as psA, \
             tc.tile_pool(name="psB", bufs=1, space="PSUM") as psB, \
             tc.tile_pool(name="ps4", bufs=1, space="PSUM") as ps4:

            w1t = cp.tile([8, 120], dt.float32, tag="w1t")
            nc.sync.dma_start(w1t[:], w1s[:])
            w2t = cp.tile([120, 120], dt.float32, tag="w2t")
            nc.sync.dma_start(w2t[:], w2s[:])
            w3t = cp.tile([120, 120], dt.float32, tag="w3t")
            nc.sync.dma_start(w3t[:], w3s[:])
            bqt = cp.tile([120, 2048], dt.float32, tag="bqt")
            nc.sync.dma_start(bqt[:], bqd[:])
            b1t = cp.tile([120, 1], dt.float32, tag="b1t")
            nc.sync.dma_start(b1t[:], b1d[:])
            b2t = cp.tile([120, 1], dt.float32, tag="b2t")
            nc.sync.dma_start(b2t[:], b2d[:])
            b3t = cp.tile([120, 1], dt.float32, tag="b3t")
            nc.sync.dma_start(b3t[:], b3d[:])
            b4t = cp.tile([128, 1], dt.float32, tag="b4t")
            nc.sync.dma_start(b4t[:], b4d[:])

            # persistent state tiles
            cea = sp.tile([128, 68], dt.float32, tag="cea")
            ceb = sp.tile([128, 68], dt.float32, tag="ceb")
            ce2 = sp.tile([128, 68], dt.float32, tag="ce2")
            ce3 = sp.tile([128, 68], dt.float32, tag="ce3")
            ce4 = sp.tile([128, 68], dt.float32, tag="ce4")
            kts = [sp.tile([128, 66], dt.float32, tag=f"kt{i}", name=f"kt{i}")
                   for i in range(4)]
            sc = sp.tile([128, 2], dt.float32, tag="sc")

            for ce in (cea, ceb, ce2, ce3, ce4):
                nc.vector.memset(ce[0:1, 0:1], BC00)

            nc.sync.dma_start(cea[:, 1:67], c0s.rearrange("(p j) -> p j", j=WT))

            coefs = [None, float(cf2), float(cf2), float(cf4)]

            for step in range(nsteps):
                base = cea if step % 2 == 0 else ceb
                nxt = ceb if step % 2 == 0 else cea
                for s in range(4):
                    ce = (base, ce2, ce3, ce4)[s]
                    if s > 0:
                        # c_s = c_base + coef * k~_{s-1}
                        nc.vector.scalar_tensor_tensor(
                            ce[:, 1:67], kts[s - 1][:], coefs[s],
                            base[:, 1:67], OP.mult, OP.add)
                    # Cauchy right ghost, computed on the full column; the
                    # halo DMA then overwrites rows 0..126, leaving only
                    # row 127 = DDX*(c[L-2] - c[L-1]) in place.
                    nc.vector.tensor_sub(sc[:, 0:1],
                                         ce[:, 65:66], ce[:, 66:67])
                    nc.vector.tensor_scalar_mul(ce[:, 67:68],
                                                sc[:, 0:1], float(DDX))
                    # halo columns (partition-shifted copies)
                    nc.sync.dma_start(ce[1:128, 0:1], ce[0:127, 66:67])
                    nc.sync.dma_start(ce[0:127, 67:68], ce[1:128, 1:2])

                    # fold tall -> packed
                    x = wp.tile([8, 1056], dt.float32, tag="x")
                    for q in range(16):
                        nc.sync.dma_start(x[0:8, 66 * q:66 * (q + 1)],
                                          ce[q:128:16, 1:67])

                    # L1: z1[120, 264*4 @512 stride] = w1s^T x
                    zA = psA.tile([120, 2048], dt.float32, tag="zA")
                    for c in range(4):
                        nc.tensor.matmul(zA[0:120, 512 * c:512 * c + 264],
                                         mmdt(w1t[:]),
                                         mmdt(x[0:8, 264 * c:264 * (c + 1)]),
                                         start=True, stop=True)
                    h1 = wp.tile([120, 1056], dt.float32, tag="h1")
                    zAv = zA[:].rearrange("p (c k) -> p c k", k=512)[:, :, 0:264]
                    h1v = h1[:].rearrange("p (c k) -> p c k", k=264)
                    nc.scalar.activation(h1v, zAv, AF.Tanh, bias=b1t[:])

                    # L2
                    zB = psB.tile([120, 1536], dt.float32, tag="zB")
                    for c in range(3):
                        nc.tensor.matmul(zB[0:120, 512 * c:512 * c + 352],
                                         mmdt(w2t[:]),
                                         mmdt(h1[:, 352 * c:352 * (c + 1)]),
                                         start=True, stop=True)
                    h2 = wp.tile([120, 1056], dt.float32, tag="h2")
                    zBv = zB[:].rearrange("p (c k) -> p c k", k=512)[:, :, 0:352]
                    h2v = h2[:].rearrange("p (c k) -> p c k", k=352)
                    nc.scalar.activation(h2v, zBv, AF.Tanh, bias=b2t[:])

                    # L3 (reuse zA banks)
                    zC = psA.tile([120, 2048], dt.float32, tag="zA")
                    for c in range(3):
                        nc.tensor.matmul(zC[0:120, 512 * c:512 * c + 352],
                                         mmdt(w3t[:]),
                                         mmdt(h2[:, 352 * c:352 * (c + 1)]),
                                         start=True, stop=True)
                    h3 = wp.tile([120, 1056], dt.float32, tag="h3")
                    zCv = zC[:].rearrange("p (c k) -> p c k",
                                          k=512)[:, 0:3, 0:352]
                    h3v = h3[:].rearrange("p (c k) -> p c k", k=352)
                    nc.scalar.activation(h3v, zCv, AF.Tanh, bias=b3t[:])

                    # L4 fused unfold: 16 scatter matmuls -> z4 tall [128, 66]
                    z4 = ps4.tile([128, 66], dt.float32, tag="z4")
                    for q in range(16):
                        nc.tensor.matmul(z4[:],
                                         bqt[:, 128 * q:128 * (q + 1)],
                                         h3[:, 66 * q:66 * (q + 1)],
                                         start=(q == 0), stop=(q == 15))
                    ret = wp.tile([128, 66], dt.float32, tag="ret")
                    nc.scalar.activation(ret[:], z4[:], AF.Sigmoid, bias=b4t[:])

                    # stencil: k~ = ret * (left + right - 2c)
                    t1 = wp.tile([128, 66], dt.float32, tag="t1")
                    nc.vector.tensor_add(t1[:], ce[:, 0:66], ce[:, 2:68])
                    lap = wp.tile([128, 66], dt.float32, tag="lap")
                    nc.vector.scalar_tensor_tensor(lap[:], ce[:, 1:67], -2.0,
                                                   t1[:], OP.mult, OP.add)
                    nc.vector.tensor_mul(kts[s][:], ret[:], lap[:])

                # c_new = c + gam*(k1 + 2k2 + 2k3 + k4)
                u1 = wp.tile([128, 66], dt.float32, tag="u1")
                nc.vector.tensor_add(u1[:], kts[0][:], kts[3][:])
                u2 = wp.tile([128, 66], dt.float32, tag="u2")
                nc.vector.tensor_add(u2[:], kts[1][:], kts[2][:])
                u3 = wp.tile([128, 66], dt.float32, tag="u3")
                nc.vector.scalar_tensor_tensor(u3[:], u2[:], 2.0, u1[:],
                                               OP.mult, OP.add)
                nc.vector.scalar_tensor_tensor(nxt[:, 1:67], u3[:], float(gam),
                                               base[:, 1:67], OP.mult, OP.add)
                nc.sync.dma_start(
                    outs[step].rearrange("(p j) -> p j", j=WT), nxt[:, 1:67])

    nc.compile()
    return nc


def _prep_consts(W1, b1, W2, b2, W3, b3, W4, b4, p_exp):
    scale = np.float32(10.0) ** p_exp.astype(np.float32)[0]
    w1s = np.zeros((8, 120), np.float32)
    w2s = np.zeros((120, 120), np.float32)
    w3s = np.zeros((120, 120), np.float32)
    bq = np.zeros((120, 2048), np.float32)
    w1sc = (W1.astype(np.float32)[0] * scale)  # [15]
    for r in range(8):
        w1s[r, 15 * r:15 * r + 15] = w1sc
        w2s[15 * r:15 * r + 15, 15 * r:15 * r + 15] = W2
        w3s[15 * r:15 * r + 15, 15 * r:15 * r + 15] = W3
        for q in range(16):
            bq[15 * r:15 * r + 15, 128 * q + 16 * r + q] = W4[:, 0]
    b1r = np.tile(b1.astype(np.float32), 8)[:, None]
    b2r = np.tile(b2.astype(np.float32), 8)[:, None]
    b3r = np.tile(b3.astype(np.float32), 8)[:, None]
    b4r = np.full((128, 1), np.asarray(b4, np.float32).reshape(-1)[0],
                  np.float32)
    return w1s, w2s, w3s, bq, b1r, b2r, b3r, b4r


def _slabs(c0):
    """Per-core ghost-extended slabs and the offset of the owned region."""
    slabs, offs = [], []
    for m in range(NCORES):
        if m == 0:
            s0 = 0
        elif m == NCORES - 1:
            s0 = N_FULL - L
        else:
            s0 = m * OWN - GHOST
        slabs.append(c0[s0:s0 + L])
        offs.append(m * OWN - s0)
    return slabs, offs


def kernel(c0, t, W1, b1, W2, b2, W3, b3, W4, b4, p_exp):
    from concourse.bass_utils import run_bass_kernel_spmd

    c0 = np.asarray(c0, np.float32)
    t = np.asarray(t, np.float32)
    nsteps = t.shape[0] - 1
    dts = t[1:] - t[:-1]
    assert np.all(dts == dts[0]), "constant dt assumed"
    dtv = np.float32(dts[0])

    cf2 = np.float32(np.float32(0.5) * dtv * D0)
    cf4 = np.float32(dtv * D0)
    gam = np.float32((dtv / np.float32(6.0)) * D0)

    use_f32r = os.environ.get("KERNEL_F32R", "0") == "1"
    key = (nsteps, float(dtv), use_f32r)
    if key not in _CACHE:
        _CACHE[key] = _build(nsteps, cf2, cf4, gam, use_f32r)
    nc = _CACHE[key]

    w1s, w2s, w3s, bq, b1r, b2r, b3r, b4r = _prep_consts(
        np.asarray(W1), np.asarray(b1), np.asarray(W2), np.asarray(b2),
        np.asarray(W3), np.asarray(b3), np.asarray(W4), np.asarray(b4),
        np.asarray(p_exp))

    slabs, offs = _slabs(c0)
    in_maps = [dict(c0s=slabs[m], w1s=w1s, w2s=w2s, w3s=w3s, bq=bq,
                    b1=b1r, b2=b2r, b3=b3r, b4=b4r) for m in range(NCORES)]

    res = run_bass_kernel_spmd(nc, in_maps, list(range(NCORES)))

    out = np.empty((nsteps + 1, N_FULL), np.float32)
    out[0] = c0
    for m in range(NCORES):
        o = offs[m]
        out[1:, m * OWN:(m + 1) * OWN] = res.results[m]["outs"][:, o:o + OWN]
    return out


# revision 11
# speedup vs baseline: 7.6808x; 7.6808x over previous
"""Trainium2 Bass kernel for nn_ConcentrationPredictor.

Reaction-diffusion RK4 integrator: dc/dt = D0 * ret(c) * lap(c), where
ret(c) = sigmoid(MLP(tanh x3)) evaluated pointwise, lap is a 1-D 3-point
stencil with a Dirichlet-flux left BC and Cauchy right BC. 33 time points,
32 RK4 steps, N = 65536 grid points.

Distribution: 1-D domain decomposition over 8 NeuronCores with 128-point
ghost zones on each side. RK4 widens the dependency stencil by 4 points per
step, so 32 steps * 4 = 128 ghost points allow each core to integrate its
slab completely independently -- zero inter-core communication. Each core's
slab is 8192 + 2*128 = 8448 points (edge cores get one-sided ghosts). The
fake boundary condition applied at interior slab edges contaminates at most
4*t points by step t, never reaching the owned region.

Per-core layout: the slab lives "tall" as [128 partitions x 66] f32 (grid
index g = 66*p + j) for the stencil (free-dim shifts) and DVE/ACT work at
full 128-lane occupancy. The MLP (1-15-15-15-1) runs "packed" as a
block-diagonal batch of 8 MLPs across 120 partitions: block r handles
chunks 16r..16r+15 of the grid, so a [120,120] block-diag matmul advances
8192+ points per 15-wide layer. Fold tall->packed is 16 partition-strided
SBUF->SBUF DMAs; unfold packed->tall is fused into the last MLP layer as 16
accumulating matmuls with scatter weight matrices (B_q), so the sigmoid
lands directly in tall layout.
"""

import os
import sys

sys.path.insert(0, "/opt/trn_rl_repo")

import numpy as np

N_FULL = 65536
T_FULL = 33
NCORES = 8
GHOST = 128
L = 8448           # slab length per core
WT = 66            # tall width  (L / 128)
BW = 1056          # packed block width (L / 8)
OWN = N_FULL // NCORES  # 8192

DX = np.float32(0.04)
D0 = np.float32(0.0005 / 0.04 ** 2)   # 0.3125
DDX = np.float32(D0 * DX)             # Cauchy BC factor
BC00 = 1.0

_CACHE = {}


def _build(nsteps, cf2, cf4, gam, use_f32r=True):
    import concourse.bacc as bacc
    import concourse.tile as tile
    import concourse.mybir as mybir

    dt = mybir.dt
    AF = mybir.ActivationFunctionType
    OP = mybir.AluOpType

    nc = bacc.Bacc("TRN2", target_bir_lowering=False, debug=False,
                   num_devices=NCORES)

    c0s = nc.dram_tensor("c0s", [L], dt.float32, kind="ExternalInput").ap()
    w1s = nc.dram_tensor("w1s", [8, 120], dt.float32, kind="ExternalInput").ap()
    w2s = nc.dram_tensor("w2s", [120, 120], dt.float32, kind="ExternalInput").ap()
    w3s = nc.dram_tensor("w3s", [120, 120], dt.float32, kind="ExternalInput").ap()
    bqd = nc.dram_tensor("bq", [120, 2048], dt.float32, kind="ExternalInput").ap()
    b1d = nc.dram_tensor("b1", [120, 1], dt.float32, kind="ExternalInput").ap()
    b2d = nc.dram_tensor("b2", [120, 1], dt.float32, kind="ExternalInput").ap()
    b3d = nc.dram_tensor("b3", [120, 1], dt.float32, kind="ExternalInput").ap()
    b4d = nc.dram_tensor("b4", [128, 1], dt.float32, kind="ExternalInput").ap()
    outs = nc.dram_tensor("outs", [nsteps, L], dt.float32,
                          kind="ExternalOutput").ap()

    def mmdt(ap):
        return ap.bitcast(dt.float32r) if use_f32r else ap

    with tile.TileContext(nc) as tc:
        with tc.tile_pool(name="consts", bufs=1) as cp, \
             tc.tile_pool(name="state", bufs=1) as sp, \
             tc.tile_pool(name="work", bufs=2) as wp, \
             tc.tile_pool(name="psA", bufs=1, space="PSUM") as psA, \
             tc.tile_pool(name="psB", bufs=1, space="PSUM") as psB, \
             tc.tile_pool(name="ps4", bufs=1, space="PSUM") as ps4:

            w1t = cp.tile([8, 120], dt.float32, tag="w1t")
            nc.sync.dma_start(w1t[:], w1s[:])
            w2t = cp.tile([120, 120], dt.float32, tag="w2t")
            nc.sync.dma_start(w2t[:], w2s[:])
            w3t = cp.tile([120, 120], dt.float32, tag="w3t")
            nc.sync.dma_start(w3t[:], w3s[:])
            bqt = cp.tile([120, 2048], dt.float32, tag="bqt")
            nc.sync.dma_start(bqt[:], bqd[:])
            b1t = cp.tile([120, 1], dt.float32, tag="b1t")
            nc.sync.dma_start(b1t[:], b1d[:])
            b2t = cp.tile([120, 1], dt.float32, tag="b2t")
            nc.sync.dma_start(b2t[:], b2d[:])
            b3t = cp.tile([120, 1], dt.float32, tag="b3t")
            nc.sync.dma_start(b3t[:], b3d[:])
            b4t = cp.tile([128, 1], dt.float32, tag="b4t")
            nc.sync.dma_start(b4t[:], b4d[:])

            # persistent state tiles
            cea = sp.tile([128, 68], dt.float32, tag="cea")
            ceb = sp.tile([128, 68], dt.float32, tag="ceb")
            ce2 = sp.tile([128, 68], dt.float32, tag="ce2")
            ce3 = sp.tile([128, 68], dt.float32, tag="ce3")
            ce4 = sp.tile([128, 68], dt.float32, tag="ce4")
            kts = [sp.tile([128, 66], dt.float32, tag=f"kt{i}", name=f"kt{i}")
                   for i in range(4)]
            sc = sp.tile([128, 2], dt.float32, tag="sc")

            for ce in (cea, ceb, ce2, ce3, ce4):
                nc.vector.memset(ce[0:1, 0:1], BC00)

            nc.sync.dma_start(cea[:, 1:67], c0s.rearrange("(p j) -> p j", j=WT))

            coefs = [None, float(cf2), float(cf2), float(cf4)]

            for step in range(nsteps):
                base = cea if step % 2 == 0 else ceb
                nxt = ceb if step % 2 == 0 else cea
                for s in range(4):
                    ce = (base, ce2, ce3, ce4)[s]
                    if s > 0:
                        # c_s = c_base + coef * k~_{s-1}
                        nc.vector.scalar_tensor_tensor(
                            ce[:, 1:67], kts[s - 1][:], coefs[s],
                            base[:, 1:67], OP.mult, OP.add)
                    # Cauchy right ghost, computed on the full column; the
                    # halo DMA then overwrites rows 0..126, leaving only
                    # row 127 = DDX*(c[L-2] - c[L-1]) in place.
                    nc.vector.tensor_sub(sc[:, 0:1],
                                         ce[:, 65:66], ce[:, 66:67])
                    nc.vector.tensor_scalar_mul(ce[:, 67:68],
                                                sc[:, 0:1], float(DDX))
                    # halo columns (partition-shifted copies)
                    nc.sync.dma_start(ce[1:128, 0:1], ce[0:127, 66:67])
                    nc.sync.dma_start(ce[0:127, 67:68], ce[1:128, 1:2])

                    # fold tall -> packed
                    x = wp.tile([8, 1056], dt.float32, tag="x")
                    for q in range(16):
                        nc.sync.dma_start(x[0:8, 66 * q:66 * (q + 1)],
                                          ce[q:128:16, 1:67])

                    # L1: z1[120, 264*4 @512 stride] = w1s^T x
                    zA = psA.tile([120, 2048], dt.float32, tag="zA")
                    for c in range(4):
                        nc.tensor.matmul(zA[0:120, 512 * c:512 * c + 264],
                                         mmdt(w1t[:]),
                                         mmdt(x[0:8, 264 * c:264 * (c + 1)]),
                                         start=True, stop=True)
                    h1 = wp.tile([120, 1056], dt.float32, tag="h1")
                    zAv = zA[:].rearrange("p (c k) -> p c k", k=512)[:, :, 0:264]
                    h1v = h1[:].rearrange("p (c k) -> p c k", k=264)
                    nc.scalar.activation(h1v, zAv, AF.Tanh, bias=b1t[:])

                    # L2
                    zB = psB.tile([120, 1536], dt.float32, tag="zB")
                    for c in range(3):
                        nc.tensor.matmul(zB[0:120, 512 * c:512 * c + 352],
                                         mmdt(w2t[:]),
                                         mmdt(h1[:, 352 * c:352 * (c + 1)]),
                                         start=True, stop=True)
                    h2 = wp.tile([120, 1056], dt.float32, tag="h2")
                    zBv = zB[:].rearrange("p (c k) -> p c k", k=512)[:, :, 0:352]
                    h2v = h2[:].rearrange("p (c k) -> p c k", k=352)
                    nc.scalar.activation(h2v, zBv, AF.Tanh, bias=b2t[:])

                    # L3 (reuse zA banks)
                    zC = psA.tile([120, 2048], dt.float32, tag="zA")
                    for c in range(3):
                        nc.tensor.matmul(zC[0:120, 512 * c:512 * c + 352],
                                         mmdt(w3t[:]),
                                         mmdt(h2[:, 352 * c:352 * (c + 1)]),
                                         start=True, stop=True)
                    h3 = wp.tile([120, 1056], dt.float32, tag="h3")
                    zCv = zC[:].rearrange("p (c k) -> p c k",
                                          k=512)[:, 0:3, 0:352]
                    h3v = h3[:].rearrange("p (c k) -> p c k", k=352)
                    nc.scalar.activation(h3v, zCv, AF.Tanh, bias=b3t[:])

                    # L4 fused unfold: 16 scatter matmuls -> z4 tall [128, 66]
                    z4 = ps4.tile([128, 66], dt.float32, tag="z4")
                    for q in range(16):
                        nc.tensor.matmul(z4[:],
                                         bqt[:, 128 * q:128 * (q + 1)],
                                         h3[:, 66 * q:66 * (q + 1)],
                                         start=(q == 0), stop=(q == 15))
                    ret = wp.tile([128, 66], dt.float32, tag="ret")
                    nc.scalar.activation(ret[:], z4[:], AF.Sigmoid, bias=b4t[:])

                    # stencil: k~ = ret * (left + right - 2c)
                    t1 = wp.tile([128, 66], dt.float32, tag="t1")
                    nc.vector.tensor_add(t1[:], ce[:, 0:66], ce[:, 2:68])
                    lap = wp.tile([128, 66], dt.float32, tag="lap")
                    nc.vector.scalar_tensor_tensor(lap[:], ce[:, 1:67], -2.0,
                                                   t1[:], OP.mult, OP.add)
                    nc.vector.tensor_mul(kts[s][:], ret[:], lap[:])

                # c_new = c + gam*(k1 + 2k2 + 2k3 + k4)
                u1 = wp.tile([128, 66], dt.float32, tag="u1")
                nc.vector.tensor_add(u1[:], kts[0][:], kts[3][:])
                u2 = wp.tile([128, 66], dt.float32, tag="u2")
                nc.vector.tensor_add(u2[:], kts[1][:], kts[2][:])
                u3 = wp.tile([128, 66], dt.float32, tag="u3")
                nc.vector.scalar_tensor_tensor(u3[:], u2[:], 2.0, u1[:],
                                               OP.mult, OP.add)
                nc.vector.scalar_tensor_tensor(nxt[:, 1:67], u3[:], float(gam),
                                               base[:, 1:67], OP.mult, OP.add)
                nc.sync.dma_start(
                    outs[step].rearrange("(p j) -> p j", j=WT), nxt[:, 1:67])

    nc.compile()
    return nc


def _prep_consts(W1, b1, W2, b2, W3, b3, W4, b4, p_exp):
    scale = np.float32(10.0) ** p_exp.astype(np.float32)[0]
    w1s = np.zeros((8, 120), np.float32)
    w2s = np.zeros((120, 120), np.float32)
    w3s = np.zeros((120, 120), np.float32)
    bq = np.zeros((120, 2048), np.float32)
    w1sc = (W1.astype(np.float32)[0] * scale)  # [15]
    for r in range(8):
        w1s[r, 15 * r:15 * r + 15] = w1sc
        w2s[15 * r:15 * r + 15, 15 * r:15 * r + 15] = W2
        w3s[15 * r:15 * r + 15, 15 * r:15 * r + 15] = W3
        for q in range(16):
            bq[15 * r:15 * r + 15, 128 * q + 16 * r + q] = W4[:, 0]
    b1r = np.tile(b1.astype(np.float32), 8)[:, None]
    b2r = np.tile(b2.astype(np.float32), 8)[:, None]
    b3r = np.tile(b3.astype(np.float32), 8)[:, None]
    b4r = np.full((128, 1), np.asarray(b4, np.float32).reshape(-1)[0],
                  np.float32)
    return w1s, w2s, w3s, bq, b1r, b2r, b3r, b4r


def _slabs(c0):
    """Per-core ghost-extended slabs and the offset of the owned region."""
    slabs, offs = [], []
    for m in range(NCORES):
        if m == 0:
            s0 = 0
        elif m == NCORES - 1:
            s0 = N_FULL - L
        else:
            s0 = m * OWN - GHOST
        slabs.append(c0[s0:s0 + L])
        offs.append(m * OWN - s0)
    return slabs, offs


def _make_runner(nc):
    """Build a persistent jitted 8-core executor for the compiled Bass
    program (mirrors bass2jax.run_bass_via_pjrt, but reusable across calls
    so the NEFF executable is traced/loaded once)."""
    import jax
    import numpy as _np
    from jax.sharding import Mesh, PartitionSpec
    from jax.experimental.shard_map import shard_map
    import concourse.mybir as mybir
    from concourse import bass2jax

    bass2jax.install_neuronx_cc_hook()

    partition_name = (nc.partition_id_tensor.name
                      if nc.partition_id_tensor else None)
    in_names, out_names, out_avals, zero_outs = [], [], [], []
    for alloc in nc.m.functions[0].allocations:
        if not isinstance(alloc, mybir.MemoryLocationSet):
            continue
        name = alloc.memorylocations[0].name
        if alloc.kind == "ExternalInput":
            if name != partition_name:
                in_names.append(name)
        elif alloc.kind == "ExternalOutput":
            out_names.append(name)
            shape = tuple(alloc.tensor_shape)
            dtype = mybir.dt.np(alloc.dtype)
            out_avals.append(jax.core.ShapedArray(shape, dtype))
            zero_outs.append(_np.zeros(shape, dtype))
    n_params = len(in_names)
    n_outs = len(out_avals)
    all_in_names = list(in_names) + list(out_names)
    if partition_name is not None:
        all_in_names.append(partition_name)

    def _body(*args):
        operands = list(args)
        if partition_name is not None:
            operands.append(bass2jax.partition_id_tensor())
        outs = bass2jax._bass_exec_p.bind(
            *operands,
            out_avals=tuple(out_avals),
            in_names=tuple(all_in_names),
            out_names=tuple(out_names),
            lowering_input_output_aliases=(),
            sim_require_finite=True,
            sim_require_nnan=True,
            nc=nc,
        )
        return tuple(outs)

    devices = jax.devices()[:NCORES]
    mesh = Mesh(_np.asarray(devices), ("core",))
    in_specs = (PartitionSpec("core"),) * (n_params + n_outs)
    out_specs = (PartitionSpec("core"),) * n_outs
    donate = tuple(range(n_params, n_params + n_outs))
    sharded = jax.jit(
        shard_map(_body, mesh=mesh, in_specs=in_specs, out_specs=out_specs,
                  check_rep=False),
        donate_argnums=donate, keep_unused=True)

    def run(in_maps):
        per_core = [[_np.asarray(m[n]) for n in in_names] for m in in_maps]
        concat_in = [_np.concatenate([per_core[c][i] for c in range(NCORES)],
                                     axis=0) for i in range(n_params)]
        concat_zeros = [_np.zeros((NCORES * z.shape[0], *z.shape[1:]), z.dtype)
                        for z in zero_outs]
        out_arrs = sharded(*concat_in, *concat_zeros)
        out_arrs = [_np.asarray(a) for a in out_arrs]
        return [
            {name: out_arrs[i].reshape(NCORES, *out_avals[i].shape)[c]
             for i, name in enumerate(out_names)}
            for c in range(NCORES)
        ]

    return run


def kernel(c0, t, W1, b1, W2, b2, W3, b3, W4, b4, p_exp):

    c0 = np.asarray(c0, np.float32)
    t = np.asarray(t, np.float32)
    nsteps = t.shape[0] - 1
    dts = t[1:] - t[:-1]
    assert np.all(dts == dts[0]), "constant dt assumed"
    dtv = np.float32(dts[0])

    cf2 = np.float32(np.float32(0.5) * dtv * D0)
    cf4 = np.float32(dtv * D0)
    gam = np.float32((dtv / np.float32(6.0)) * D0)

    use_f32r = os.environ.get("KERNEL_F32R", "0") == "1"
    key = (nsteps, float(dtv), use_f32r)
    if key not in _CACHE:
        nc = _build(nsteps, cf2, cf4, gam, use_f32r)
        _CACHE[key] = _make_runner(nc)
    run = _CACHE[key]

    w1s, w2s, w3s, bq, b1r, b2r, b3r, b4r = _prep_consts(
        np.asarray(W1), np.asarray(b1), np.asarray(W2), np.asarray(b2),
        np.asarray(W3), np.asarray(b3), np.asarray(W4), np.asarray(b4),
        np.asarray(p_exp))

    slabs, offs = _slabs(c0)
    in_maps = [dict(c0s=slabs[m], w1s=w1s, w2s=w2s, w3s=w3s, bq=bq,
                    b1=b1r, b2=b2r, b3=b3r, b4=b4r) for m in range(NCORES)]

    results = run(in_maps)

    out = np.empty((nsteps + 1, N_FULL), np.float32)
    out[0] = c0
    for m in range(NCORES):
        o = offs[m]
        out[1:, m * OWN:(m + 1) * OWN] = results[m]["outs"][:, o:o + OWN]
    return out
